# revision 2
# baseline (speedup 1.0000x reference)
"""Causal self-attention (b=2, n=2048, d=1024, 16 heads) on 8 NeuronCores.

Sharding: core c handles batch b = c // 4 and head group g = c % 4
(heads 4g..4g+3).  qkv weights column-sharded, proj weights row-sharded
(Megatron); each core emits a partial [2048, 1024] proj output (bf16) and
the host sums the 4 partials per batch (b_proj added host-side).

v2 design (vs f32r baseline):
  - all dense matmuls (qkv, V, proj) in bf16 (1 cyc/row, halved DMA)
  - QK^T in fp8e4 DoubleRow (0.5 cyc/row): moving operand packs
    (q8, dq8=fp8(q-q8)) on the k-subtile axis, stationary k8 is
    broadcast to both subtiles -> S = k8.(q8+dq8), error ~ k-quant only
  - AV flipped et-stationary: lhsT = et [128 ktok, 128 qtok] (bf16),
    rhs = [V|1] [128 ktok, 65] -> o/denominator [128 qtok, 65] at
    65 rows/block (the ones column gives the softmax denominator)
  - causal mask via in-place gpsimd affine_select on et (Pool engine)
  - normalize: per-token reciprocal [128,1] + DVE tensor_scalar from PSUM
  - o transposed feature-major via DMA-transpose (XBAR) for the proj
  - y emitted bf16, host sums partials in f32
  - emission: one continuous attention stream across quarters/head-pairs
    (stream boundaries overlap) with weighted filler interleaving
"""
import sys

sys.path.insert(0, "/opt/trn_rl_repo")

import numpy as np
import ml_dtypes

import concourse.bass as bass  # noqa: F401
import concourse.mybir as mybir
import concourse.tile as tile
from concourse import bacc
from concourse.bass_utils import run_bass_kernel_spmd

F32 = mybir.dt.float32
BF16 = mybir.dt.bfloat16
FP8 = mybir.dt.float8e4
Exp = mybir.ActivationFunctionType.Exp
DR = mybir.MatmulPerfMode.DoubleRow
NPBF16 = ml_dtypes.bfloat16

B = 2
N = 2048
D = 1024
NH = 16
HD = 64
NCORES = 8
GROUPS = 4                # head groups (cores per batch)
HPC = NH // GROUPS        # heads per core = 4
PAIRS = HPC // 2          # head pairs per core = 2
QS = 512                  # q_super width
NQS = N // QS             # 4
NB = N // 128             # 16 token blocks
CCH = D // 128            # 8 contraction chunks
LOOKAHEAD = 3

# o PSUM layout: 8 blocks of 65 f32 packed in 2 banks without any block
# crossing a 2048B boundary. block index idx = 2*b + h.
O_OFF = [65 * i for i in range(7)] + [512]

_CACHE = {}


def _build():
    nc = bacc.Bacc("TRN2", target_bir_lowering=False, debug=False,
                   num_devices=NCORES)
    xT = nc.dram_tensor("xT", [D, N], BF16, kind="ExternalInput").ap()
    W = nc.dram_tensor("W", [D, 768], BF16, kind="ExternalInput").ap()
    Wp = nc.dram_tensor("Wp", [256, D], BF16, kind="ExternalInput").ap()
    biasqk = nc.dram_tensor("biasqk", [128, 4], F32, kind="ExternalInput").ap()
    vbias = nc.dram_tensor("vbias", [128, 256], F32, kind="ExternalInput").ap()
    y = nc.dram_tensor("y", [N, D], BF16, kind="ExternalOutput").ap()

    with tile.TileContext(nc) as tc:
        with (
            tc.tile_pool(name="persist", bufs=1) as pp,
            tc.tile_pool(name="xtq_pool", bufs=3) as xtq_pool,
            tc.tile_pool(name="et_pool", bufs=24) as et_pool,
            tc.tile_pool(name="work", bufs=3) as work,
            tc.tile_pool(name="ysb_pool", bufs=3) as ysb_pool,
            tc.tile_pool(name="mm", bufs=2, space="PSUM") as mm,
            tc.tile_pool(name="spool", bufs=2, space="PSUM") as spool,
            tc.tile_pool(name="opool", bufs=1, space="PSUM") as opool,
        ):
            # ---- persistent tiles ----
            W_sb = pp.tile([128, CCH, 768], BF16)
            Wp_sb = pp.tile([128, 2, D], BF16)
            bqk_sb = pp.tile([128, 4], F32)
            vbias_sb = pp.tile([128, 256], F32)
            q8 = pp.tile([128, PAIRS, 2, N], FP8)   # (hp, sub(q8/dq8), tok)
            k8 = pp.tile([128, PAIRS, N], FP8)      # (hp, tok)
            vaug = pp.tile([128, NB, HPC, 65], BF16)  # (kblk, head, hd|1)

            W_r = W.rearrange("(c p) f -> p c f", p=128)
            Wp_r = Wp.rearrange("(c p) f -> p c f", p=128)
            xT_r = xT.rearrange("(c p) n -> p c n", p=128)
            y_r = y.rearrange("(t p) f -> t p f", p=128)

            # ones column of [V|1]
            nc.vector.memset(vaug[:, :, :, 64], 1.0)
            ident = pp.tile([128, 128], BF16)
            nc.gpsimd.memset(ident[:], 1.0)
            nc.gpsimd.affine_select(
                out=ident[:], in_=ident[:],
                compare_op=mybir.AluOpType.is_equal, fill=0.0,
                base=0, pattern=[[1, 128]], channel_multiplier=-1,
            )

            def fetch_xq(q, eng=None):
                t0, t1 = QS * q, QS * (q + 1)
                xq = xtq_pool.tile([128, CCH, QS], BF16, tag="xq",
                                   name=f"xq{q}")
                (eng or nc.sync).dma_start(xq[:], xT_r[:, :, t0:t1])
                return xq

            # =========================================================
            # filler machinery: weighted queue of (weight, tag, thunk)
            # =========================================================
            # ---- qkv m-tile unit: one complete m-tile, self-contained ----
            def make_mtile_units(qtr, xq):
                ts, te = QS * qtr, QS * (qtr + 1)

                def mm_unit(m):
                    def emit():
                        ps = mm.tile([128, QS], F32, tag="mm",
                                     name=f"qk{qtr}{m}")
                        for ci in range(CCH):
                            nc.tensor.matmul(
                                ps[:],
                                W_sb[:, ci, 128 * m: 128 * (m + 1)],
                                xq[:, ci, :],
                                start=(ci == 0),
                                stop=(ci == CCH - 1),
                            )
                        hp = m % 2
                        if m < 2:  # q: biased fp8 + residual
                            nc.vector.tensor_scalar_add(
                                q8[:, hp, 0, ts:te], ps[:],
                                bqk_sb[:, m: m + 1],
                            )
                            nc.vector.tensor_sub(
                                q8[:, hp, 1, ts:te], ps[:],
                                q8[:, hp, 0, ts:te],
                            )
                        else:      # k: biased fp8
                            nc.vector.tensor_scalar_add(
                                k8[:, hp, ts:te], ps[:],
                                bqk_sb[:, m: m + 1],
                            )
                    return emit

                return [mm_unit(m) for m in (0, 2, 1, 3)]

            def make_v_unit(qtr, xq, blk):
                def emit():
                    tb = 4 * qtr + blk
                    vps = mm.tile([128, 256], F32, tag="mm",
                                  name=f"v{qtr}{blk}")
                    for ci in range(CCH):
                        nc.tensor.matmul(
                            vps[:],
                            xq[:, ci, 128 * blk: 128 * (blk + 1)],
                            W_sb[:, ci, 512:768],
                            start=(ci == 0),
                            stop=(ci == CCH - 1),
                        )
                    nc.vector.tensor_add(
                        vaug[:, tb, :, 0:64],
                        vps.rearrange("p (h c) -> p h c", c=HD),
                        vbias_sb.rearrange("p (h c) -> p h c", c=HD),
                    )
                return emit

            osb_tiles = {}
            onT_tiles = {}
            pending_tp = []

            def emit_transpose(tb):
                """transpose o -> oT on the PE (tiny) + one DVE copyback"""
                osb = osb_tiles.pop(tb)
                osb_flat = osb.rearrange("p a b -> p (a b)")
                tps = mm.tile([128, 2, 128], BF16, tag="mm",
                              name=f"tp{tb}")
                for c in range(2):
                    nc.tensor.transpose(
                        tps[:, c, :], osb_flat[:, 128 * c: 128 * (c + 1)],
                        ident[:],
                    )
                ont = work.tile([128, 2, 128], BF16, tag="onT", bufs=12,
                                name=f"onT{tb}")
                nc.vector.tensor_copy(ont[:], tps[:])
                onT_tiles[tb] = ont

            def make_proj_unit(tb):
                def emit():
                    ont = onT_tiles.pop(tb)
                    ysb = ysb_pool.tile([128, 2, QS], BF16, tag="ysb",
                                        name=f"ysb{tb}")
                    for nh in range(2):
                        yps = mm.tile([128, QS], F32, tag="mm",
                                      name=f"y{tb}{nh}")
                        for c in range(2):
                            nc.tensor.matmul(
                                yps[:],
                                ont[:, c, :],
                                Wp_sb[:, c, QS * nh: QS * (nh + 1)],
                                start=(c == 0),
                                stop=(c == 1),
                            )
                        if tb >= 12:
                            nc.scalar.copy(ysb[:, nh, :], yps[:])
                            nc.sync.dma_start(
                                y_r[tb][:, QS * nh: QS * (nh + 1)],
                                ysb[:, nh, :])
                        else:
                            nc.vector.tensor_copy(ysb[:, nh, :], yps[:])
                    if tb < 12:
                        nc.sync.dma_start(y_r[tb],
                                          ysb.rearrange("p a b -> p (a b)"))
                return emit

            # =========================================================
            # attention stream for one (quarter, head pair): a list of
            # step-thunks.  main[t]: QK_t (+ AV_{t-L}); drain: AV-only.
            # =========================================================
            # single persistent o accumulator: subtile deps let stream N+1's
            # AV into block (h,b) begin as soon as stream N normalized it
            o_all = opool.tile([128, 1024], F32, tag="o", name="o_all")

            def attention_stream(j, hp):
                n_i = 4 * j + 4
                state = {}

                def setup():
                    state["o"] = o_all
                    state["ets"] = {}

                def emit_qk(i):
                    t = i - 4 * j
                    qs0 = 0 if t < 0 else 128 * t
                    sps = spool.tile([128, 2, QS], F32, tag="s",
                                     name=f"s{j}{hp}{i}")
                    for h in range(2):
                        pb = 64 * h
                        lhsT = (k8[pb:pb + 64, hp, 128 * i:128 * (i + 1)]
                                .unsqueeze(1).broadcast_to([64, 2, 128]))
                        nc.tensor.matmul(
                            sps[:, h, qs0:],
                            lhsT,
                            q8[pb:pb + 64, hp, :,
                               QS * j + qs0: QS * (j + 1)],
                            start=True, stop=True, perf_mode=DR,
                        )
                    et = et_pool.tile([128, 2, QS], BF16, tag="et",
                                      name=f"et{j}{hp}{i}")
                    nc.scalar.activation(et[:, :, qs0:], sps[:, :, qs0:],
                                         Exp, scale=0.125)
                    if t >= 0:
                        nc.gpsimd.affine_select(
                            out=et[:, :, qs0:qs0 + 128],
                            in_=et[:, :, qs0:qs0 + 128],
                            compare_op=mybir.AluOpType.is_ge,
                            fill=0.0,
                            base=0,
                            pattern=[[0, 2], [1, 128]],
                            channel_multiplier=-1,
                        )
                    state["ets"][i] = et

                def emit_block_group(b):
                    """full accumulation for q-block b as back-to-back
                    groups (one open PSUM group per bank at a time)"""
                    tb = 4 * j + b
                    o = state["o"]
                    for h in range(2):
                        off = O_OFF[2 * b + h]
                        for i in range(tb + 1):
                            nc.tensor.matmul(
                                o[:, off:off + 65],
                                state["ets"][i][:, h, 128 * b:128 * (b + 1)],
                                vaug[:, i, 2 * hp + h, :],
                                start=(i == 0), stop=(i == tb),
                            )
                    emit_norm(b)

                def emit_norm(b):
                    tb = 4 * j + b
                    o = state["o"]
                    if hp == 0:
                        osb_tiles[tb] = work.tile(
                            [128, HPC, HD], BF16, tag="osb", bufs=12,
                            name=f"osb{tb}")
                    osb = osb_tiles[tb]
                    for h in range(2):
                        off = O_OFF[2 * b + h]
                        rd = work.tile([128, 1], F32, tag="rd", bufs=4,
                                       name=f"rd{j}{hp}{b}{h}")
                        nc.vector.reciprocal(rd[:], o[:, off + 64:off + 65])
                        nc.vector.tensor_scalar_mul(
                            osb[:, 2 * hp + h, :],
                            o[:, off:off + 64],
                            rd[:],
                        )
                    if hp == PAIRS - 1:
                        pending_tp.append(tb)


                def step(t, av_begin):
                    def run():
                        if t == 0:
                            setup()
                        if t < n_i:
                            emit_qk(t)
                        # q-block b's group fires once its diagonal et
                        # (i = 4j+b) has been exp'd, 2 steps later
                        bt = t - 2 - 4 * j
                        if 0 <= bt < 4:
                            emit_block_group(bt)
                    return run

                def make_steps(av_begin):
                    return [step(t, av_begin)
                            for t in range(n_i + 2)]

                return n_i, make_steps

            # =========================================================
            # initial loads + quarter 0 dense work (inline; PE idle anyway)
            # =========================================================
            xq0 = xtq_pool.tile([128, CCH, QS], BF16, tag="xq", name="xq0")
            nc.sync.dma_start(W_sb[:, 0:1, 0:512], W_r[:, 0:1, 0:512])
            nc.scalar.dma_start(xq0[:, 0:1, :], xT_r[:, 0:1, 0:QS])
            nc.sync.dma_start(xq0[:, 1:2, :], xT_r[:, 1:2, 0:QS])
            nc.scalar.dma_start(W_sb[:, 1:2, 0:512], W_r[:, 1:2, 0:512])
            nc.sync.dma_start(W_sb[:, 2:5, 0:512], W_r[:, 2:5, 0:512])
            nc.scalar.dma_start(xq0[:, 2:5, :], xT_r[:, 2:5, 0:QS])
            nc.sync.dma_start(xq0[:, 5:8, :], xT_r[:, 5:8, 0:QS])
            nc.scalar.dma_start(W_sb[:, 5:8, 0:512], W_r[:, 5:8, 0:512])
            nc.sync.dma_start(bqk_sb[:], biasqk)
            nc.sync.dma_start(W_sb[:, :, 512:768], W_r[:, :, 512:768])
            nc.sync.dma_start(vbias_sb[:], vbias)
            xqs = {0: xq0, 1: fetch_xq(1, nc.scalar)}
            nc.sync.dma_start(Wp_sb[:], Wp_r)

            for u in make_mtile_units(0, xq0):
                u()
            for blk in range(4):
                make_v_unit(0, xq0, blk)()

            # =========================================================
            # build the merged emission sequence (static schedule)
            # =========================================================
            # per-quarter slot lists: hp0 main + hp1 main, with the
            # previous stream's drain steps attached to the first L slots
            # stream boundary overlap: the previous stream's last OV steps
            # are woven one-per-slot into the next stream's first OV slots.
            # the next stream's AVs are held until past the carried norms.
            OV_WITHIN = 4
            OV_CROSS = 10
            quarter_slots = []
            carry = []
            for j in range(NQS):
                slots = []
                for hp in range(PAIRS):
                    n_i, make_steps = attention_stream(j, hp)
                    ov_in = len(carry)
                    steps = make_steps(max(LOOKAHEAD, ov_in + 1))
                    is_last = (j == NQS - 1 and hp == PAIRS - 1)
                    if is_last:
                        ov_out = 0
                    elif hp == PAIRS - 1:
                        ov_out = min(OV_CROSS, len(steps) - ov_in - 1)
                    else:
                        ov_out = min(OV_WITHIN, len(steps) - ov_in - 1)
                    own = steps[:len(steps) - ov_out]
                    for t, m in enumerate(own):
                        sl = [m]
                        if t < len(carry):
                            sl.append(carry[t])
                        slots.append(sl)
                    carry = steps[len(steps) - ov_out:]
                quarter_slots.append(slots)
            final_drain = carry

            def insert(j, pos, thunk):
                slots = quarter_slots[j]
                slots[min(len(slots) - 1, max(0, int(pos)))].append(thunk)

            # x prefetches (placed as thunks so DMA order is scheduled)
            def fetch_thunk(q):
                def real():
                    xqs[q] = fetch_xq(q)
                return real

            insert(0, 3, fetch_thunk(2))
            insert(1, 6, fetch_thunk(3))

            # qkv m-tiles for quarters 1..3 (units consume xqs[q] lazily)
            def lazy_m_units(qtr):
                holder = {}

                def unit(idx):
                    def run():
                        if "u" not in holder:
                            holder["u"] = make_mtile_units(qtr, xqs[qtr])
                        holder["u"][idx]()
                    return run
                return [unit(i) for i in range(4)]

            m1, m2, m3 = (lazy_m_units(q) for q in (1, 2, 3))
            for k, u in enumerate(m1):
                insert(0, 1 + 2 * k, u)           # q0 slots 1,3,5,7
            for k, u in enumerate(m2):
                insert(1, (3, 7, 11, 14)[k], u)
            for k, u in enumerate(m3[:2]):
                insert(1, (13, 15)[k], u)
            for k, u in enumerate(m3[2:]):
                insert(2, (2, 5)[k], u)

            # V units for quarters 1..3: before the diagonal AVs (slot
            # 4j+L of hp0), spread from slot 1
            def lazy_v_unit(qtr, blk):
                def run():
                    make_v_unit(qtr, xqs[qtr], blk)()
                return run

            for j in range(1, NQS):
                lastp = 4 * j + LOOKAHEAD - 2
                for k in range(4):
                    insert(j, 1 + k * max(1, lastp - 1) // 3,
                           lazy_v_unit(j, k))

            # proj placement: quarter 0 -> q2, quarters 1,2 -> q3,
            # quarter 3 -> interleaved into the final drain
            for b in range(4):
                insert(2, (6, 10, 14, 18)[b], make_proj_unit(0 * 4 + b))
            for b in range(4):
                insert(3, (4, 8, 12, 16)[b], make_proj_unit(4 + b))
            for b in range(4):
                insert(3, (20, 23, 26, 29)[b], make_proj_unit(8 + b))

            # ---- emit ----
            for j in range(NQS):
                for sl in quarter_slots[j]:
                    due, pending_tp[:] = pending_tp[:], []
                    for thunk in sl:
                        thunk()
                    for tb in due:
                        emit_transpose(tb)
            def flush_tp():
                for tb in pending_tp:
                    emit_transpose(tb)
                pending_tp[:] = []
            # interleave final drain with the last projs
            for d in final_drain:
                d()
            for b in range(4):
                flush_tp()
                make_proj_unit(12 + b)()
            flush_tp()

    nc.compile()
    return nc


def _host_prep(x, W_qkv, b_qkv, W_proj, b_proj):
    """Build per-core input maps."""
    x = np.asarray(x, dtype=np.float32)
    W_qkv = np.asarray(W_qkv, dtype=np.float32)
    b_qkv = np.asarray(b_qkv, dtype=np.float32)
    W_proj = np.asarray(W_proj, dtype=np.float32)

    xTs = [np.ascontiguousarray(x[b].T).astype(NPBF16) for b in range(B)]

    in_maps = []
    for c in range(NCORES):
        b, g = divmod(c, GROUPS)
        cols = slice(256 * g, 256 * (g + 1))
        Wslice = np.ascontiguousarray(
            np.concatenate(
                [W_qkv[:, cols], W_qkv[:, 1024:2048][:, cols],
                 W_qkv[:, 2048:3072][:, cols]],
                axis=1,
            )
        ).astype(NPBF16)
        bq = b_qkv[cols.start: cols.stop]
        bk = b_qkv[1024 + cols.start: 1024 + cols.stop]
        bv = b_qkv[2048 + cols.start: 2048 + cols.stop]
        biasqk = np.ascontiguousarray(
            np.stack([bq[:128], bq[128:], bk[:128], bk[128:]], axis=1)
        )
        vbias = np.ascontiguousarray(
            np.broadcast_to(bv, (128, 256))).astype(np.float32)
        Wp_slice = np.ascontiguousarray(W_proj[cols]).astype(NPBF16)
        in_maps.append(
            {
                "xT": xTs[b],
                "W": Wslice,
                "Wp": Wp_slice,
                "biasqk": biasqk,
                "vbias": vbias,
            }
        )
    return in_maps


def _make_runner(nc):
    """Build the PJRT executable once (mirrors bass2jax.run_bass_via_pjrt)
    so repeated kernel() calls skip re-tracing/compile-cache lookups."""
    import jax
    from jax.sharding import Mesh, PartitionSpec
    from jax.experimental.shard_map import shard_map

    from concourse.bass2jax import (
        _bass_exec_p,
        install_neuronx_cc_hook,
        partition_id_tensor,
    )

    install_neuronx_cc_hook()
    partition_name = (
        nc.partition_id_tensor.name if nc.partition_id_tensor else None
    )
    in_names, out_names, out_avals, zero_outs = [], [], [], []
    for alloc in nc.m.functions[0].allocations:
        if not isinstance(alloc, mybir.MemoryLocationSet):
            continue
        name = alloc.memorylocations[0].name
        if alloc.kind == "ExternalInput":
            if name != partition_name:
                in_names.append(name)
        elif alloc.kind == "ExternalOutput":
            out_names.append(name)
            shape = tuple(alloc.tensor_shape)
            dtype = mybir.dt.np(alloc.dtype)
            out_avals.append(jax.core.ShapedArray(shape, dtype))
            zero_outs.append(np.zeros(shape, dtype))
    n_params = len(in_names)
    all_in_names = in_names + out_names
    if partition_name is not None:
        all_in_names = all_in_names + [partition_name]

    def _body(*args):
        operands = list(args)
        if partition_name is not None:
            operands.append(partition_id_tensor())
        return tuple(
            _bass_exec_p.bind(
                *operands,
                out_avals=tuple(out_avals),
                in_names=tuple(all_in_names),
                out_names=tuple(out_names),
                lowering_input_output_aliases=(),
                sim_require_finite=True,
                sim_require_nnan=True,
                nc=nc,
            )
        )

    devices = jax.devices()[:NCORES]
    mesh = Mesh(np.asarray(devices), ("core",))
    in_specs = (PartitionSpec("core"),) * (n_params + len(out_names))
    out_specs = (PartitionSpec("core"),) * len(out_names)
    fn = jax.jit(
        shard_map(_body, mesh=mesh, in_specs=in_specs,
                  out_specs=out_specs, check_rep=False),
        keep_unused=True,
    )
    concat_zeros = [
        np.zeros((NCORES * z.shape[0], *z.shape[1:]), z.dtype)
        for z in zero_outs
    ]

    def run(in_maps):
        concat_in = [
            np.concatenate([np.asarray(m[name]) for m in in_maps], axis=0)
            for name in in_names
        ]
        out_arrs = fn(*concat_in, *concat_zeros)
        return [
            {
                name: np.asarray(out_arrs[i]).reshape(
                    NCORES, *out_avals[i].shape
                )[c]
                for i, name in enumerate(out_names)
            }
            for c in range(NCORES)
        ]

    return run


def kernel(x, W_qkv, b_qkv, W_proj, b_proj):
    if "nc" not in _CACHE:
        _CACHE["nc"] = _build()
        try:
            _CACHE["run"] = _make_runner(_CACHE["nc"])
        except Exception:
            _CACHE["run"] = None
    in_maps = _host_prep(x, W_qkv, b_qkv, W_proj, b_proj)
    results = None
    if _CACHE["run"] is not None:
        try:
            results = _CACHE["run"](in_maps)
        except Exception:
            results = None
    if results is None:
        # fallback: the stock path
        results = run_bass_kernel_spmd(
            _CACHE["nc"], in_maps, core_ids=list(range(NCORES))
        ).results
    out = np.zeros((B, N, D), dtype=np.float32)
    bp = np.asarray(b_proj, dtype=np.float32)
    for b in range(B):
        acc = results[4 * b]["y"].astype(np.float32)
        for g in range(1, GROUPS):
            acc = acc + results[4 * b + g]["y"].astype(np.float32)
        out[b] = acc + bp
    return out
